# revision 1
# baseline (speedup 1.0000x reference)
"""Causal self-attention with RoPE on 8 Trainium2 NeuronCores.

Sharding: tensor-parallel over heads (16 heads / 8 cores = 2 heads per
core). Each core computes q/k/v projections for its 2 heads over all
batches/tokens, runs causal flash-style attention locally, and applies
its 256-row slice of the output projection, producing a PARTIAL output
[B*T, C]. The host sums the 8 partials (the all-reduce of the row-wise
sharded Wp).

Device-side layout choices:
  - x is passed pre-transposed (xT [C, B*T]) so the contraction dim C is
    the SBUF partition dim for every projection matmul; no on-device
    transposes of x are needed.
  - Projections run token-major (psum [tok, feat]) so RoPE is a pure
    per-partition (per-token) elementwise job on DVE; q/k tiles are then
    PE-transposed to feature-major [d, tok] for attention. The transpose
    stores even-dims then odd-dims per head (a fixed permutation of d,
    identical for q and k, so scores are unchanged).
  - Scores are computed transposed (sT [k, q]) so the softmax denominator
    is a ones-matmul (partition reduction on PE) and P@V needs no
    transposes of P.
  - exp() runs without max-subtraction: scores here are ~N(0,1) after the
    1/sqrt(hd) scale, |s| < 40 by a huge margin, so fp32 exp is safe.
  - All matmuls stream as float32r (TF32-like, full PE rate at N>=256,
    measured rel err ~1.5e-4 per matmul vs 4x slower full fp32).
"""
import sys
import types

sys.path.insert(0, "/opt/trn_rl_repo")

import numpy as np

B, T, C, H, HD = 4, 2048, 2048, 16, 128
P = 128
NCORE = 8
HPC = H // NCORE            # heads per core
DLOC = HPC * HD             # local feature width (256)
NT = B * T
KT = C // P                 # 16 contraction tiles
TB = T // P                 # 16 token tiles per batch
QB = 512                    # attention q-block width
NQB = T // QB
XBLK = 256                  # xT streaming block (tokens)
SCALE = float(1.0 / np.sqrt(HD))

LAST_EXEC_NS = None
TRACE = False

_cache = {}


def _ensure_profile_shim():
    """antenv.axon_hooks is absent from the container stub; recreate it so
    run_bass_kernel_spmd(trace=True) can reach the NTFF profile hook."""
    import antenv

    if "antenv.axon_hooks" in sys.modules:
        return
    hooks = types.ModuleType("antenv.axon_hooks")
    hooks._hook = None
    hooks.set_axon_ntff_profile_hook = lambda h: setattr(hooks, "_hook", h)
    hooks.get_axon_ntff_profile_hook = lambda: hooks._hook
    sys.modules["antenv.axon_hooks"] = hooks
    antenv.axon_hooks = hooks
    try:
        from trn_agent_boot.trn_boot import _ntff_profile_via_ctypes

        hooks.set_axon_ntff_profile_hook(
            _ntff_profile_via_ctypes("/opt/axon/libaxon_pjrt.so")
        )
    except Exception:
        pass


def _split_excess_waits(nc):
    """HW instruction structs hold ONE sync wait (EventSemaphore: two), but
    Tile sometimes emits more (matmul reading two fresh tiles, the tail
    drain waiting on the whole global clock). Hoist excess waits onto
    prefix NoOps on the same engine."""
    import concourse.mybir as mybir

    uid = [0]
    for fn in nc.m.functions:
        for blk in fn.blocks:
            out = []
            for inst in blk.instructions:
                si = inst.sync_info
                waits = list(si.on_wait) if si and si.on_wait else []
                cap = 2 if inst.opcode == "EventSemaphore" else 1
                if len(waits) > cap:
                    keep = waits[-cap:]
                    for w in waits[:-cap]:
                        uid[0] += 1
                        out.append(
                            mybir.InstNoOp(
                                name=f"I-waitsplit-{uid[0]}",
                                engine=inst.engine,
                                text_hint="waitsplit",
                                sync_info=mybir.SyncInfo(on_wait=[w], on_update=[]),
                            )
                        )
                    si.on_wait = keep
                out.append(inst)
            blk.instructions = out
    return nc


def _build_nc():
    import concourse.bass as bass
    import concourse.mybir as mybir
    from concourse.masks import make_identity
    from concourse.tile import TileContext

    f32 = mybir.dt.float32
    f32r = mybir.dt.float32r
    EXP = mybir.ActivationFunctionType.Exp

    nc = bass.Bass(trn_type="TRN2", target_bir_lowering=False)
    xT = nc.dram_tensor("xT", [C, NT], f32r, kind="ExternalInput")
    wqk = nc.dram_tensor("wqk", [C, 2 * DLOC], f32r, kind="ExternalInput")
    wv = nc.dram_tensor("wv", [C, DLOC], f32r, kind="ExternalInput")
    wp = nc.dram_tensor("wp", [DLOC, C], f32r, kind="ExternalInput")
    cos2 = nc.dram_tensor("cos2", [T, P], f32, kind="ExternalInput")
    sin2 = nc.dram_tensor("sin2", [T, P], f32, kind="ExternalInput")
    tri = nc.dram_tensor("tri", [P, 640], f32r, kind="ExternalInput")
    y = nc.dram_tensor("y", [NT, C], f32, kind="ExternalOutput")

    with nc.allow_low_precision(
        reason="f32r tiles feed fp32r matmuls which round operands anyway"
    ), TileContext(nc) as tc:
        from contextlib import ExitStack
        stk = ExitStack()
        wpool = stk.enter_context(tc.tile_pool(name="wpool", bufs=1))
        cpool = stk.enter_context(tc.tile_pool(name="cpool", bufs=1))
        bpool = stk.enter_context(tc.tile_pool(name="bpool", bufs=1))
        xpool = stk.enter_context(tc.tile_pool(name="xpool", bufs=2))
        rotp = stk.enter_context(tc.tile_pool(name="rotp", bufs=2))
        tmpp = stk.enter_context(tc.tile_pool(name="tmpp", bufs=1))
        ptp = stk.enter_context(tc.tile_pool(name="ptp", bufs=2))
        ysbp = stk.enter_context(tc.tile_pool(name="ysbp", bufs=2))
        rsp = stk.enter_context(tc.tile_pool(name="rsp", bufs=1))
        rbp = stk.enter_context(tc.tile_pool(name="rbp", bufs=2))
        srp = stk.enter_context(tc.tile_pool(name="srp", bufs=1))
        drp = stk.enter_context(tc.tile_pool(name="drp", bufs=2, space="DRAM"))
        psproj = stk.enter_context(tc.tile_pool(name="psproj", bufs=3, space="PSUM"))
        psmisc = stk.enter_context(tc.tile_pool(name="psmisc", bufs=1, space="PSUM"))
        psot = stk.enter_context(tc.tile_pool(name="psot", bufs=2, space="PSUM"))
        pssc = stk.enter_context(tc.tile_pool(name="pssc", bufs=2, space="PSUM"))
        with stk:
            # ---- constants / weights ----
            wqk_sb = wpool.tile([P, KT, 2 * DLOC], f32r, tag="wqk")
            wv_sb = wpool.tile([P, KT, DLOC], f32r, tag="wv")
            wp_sb = wpool.tile([P, HPC, C], f32r, tag="wp")
            nc.sync.dma_start(out=wqk_sb, in_=wqk.rearrange("(t p) m -> p t m", p=P))
            nc.sync.dma_start(out=wv_sb, in_=wv.rearrange("(t p) m -> p t m", p=P))
            nc.sync.dma_start(out=wp_sb, in_=wp.rearrange("(h p) c -> p h c", p=P))
            cos_sb = cpool.tile([P, TB, P], f32, tag="cos")
            sin_sb = cpool.tile([P, TB, P], f32, tag="sin")
            nc.sync.dma_start(out=cos_sb, in_=cos2.rearrange("(t p) d -> p t d", p=P))
            nc.sync.dma_start(out=sin_sb, in_=sin2.rearrange("(t p) d -> p t d", p=P))
            tri_sb = cpool.tile([P, 640], f32r, tag="tri")
            nc.sync.dma_start(out=tri_sb, in_=tri[:, :])
            ident = cpool.tile([P, P], f32, tag="ident")
            make_identity(nc, ident)

            for b in range(B):
                qT = bpool.tile([P, HPC, T], f32r, tag="qT")
                kT = bpool.tile([P, HPC, T], f32r, tag="kT")
                vsb = bpool.tile([P, TB, DLOC], f32r, tag="v")
                oT = bpool.tile([P, HPC, T], f32r, tag="oT")

                # ---- phase P: qkv projection + rope + q/k transposes ----
                for blk in range(T // XBLK):
                    xt = xpool.tile([P, KT, XBLK], f32r, tag="xt")
                    col0 = b * T + blk * XBLK
                    nc.sync.dma_start(
                        out=xt,
                        in_=xT[:, col0:col0 + XBLK].rearrange("(t p) n -> p t n", p=P),
                    )
                    for st in range(XBLK // P):
                        tt = (blk * XBLK) // P + st
                        xts = xt[:, :, st * P:(st + 1) * P]
                        ps_qk = psproj.tile([P, 2 * DLOC], f32, tag="proj")
                        for ci in range(KT):
                            nc.tensor.matmul(
                                ps_qk, xts[:, ci, :], wqk_sb[:, ci, :],
                                start=(ci == 0), stop=(ci == KT - 1),
                            )
                        ps_v = psproj.tile([P, 2 * DLOC], f32, tag="proj")
                        for ci in range(KT):
                            nc.tensor.matmul(
                                ps_v[:, 0:DLOC], xts[:, ci, :], wv_sb[:, ci, :],
                                start=(ci == 0), stop=(ci == KT - 1),
                            )
                        nc.any.tensor_copy(vsb[:, tt, :], ps_v[:, 0:DLOC])
                        # rope on q (cols 0:256) and k (cols 256:512)
                        rot = rotp.tile([P, 2 * DLOC], f32, tag="rot")
                        cs = cos_sb[:, tt, :]
                        sn = sin_sb[:, tt, :]
                        for pj in range(2):
                            off = pj * DLOC
                            pair = ps_qk[:, off:off + DLOC].rearrange(
                                "p (d two) -> p d two", two=2
                            )
                            e = pair[:, :, 0]
                            o = pair[:, :, 1]
                            t1 = tmpp.tile([P, P], f32, tag="t1")
                            t2 = tmpp.tile([P, P], f32, tag="t2")
                            t3 = tmpp.tile([P, P], f32, tag="t3")
                            t4 = tmpp.tile([P, P], f32, tag="t4")
                            nc.vector.tensor_mul(t1, e, cs)
                            nc.vector.tensor_mul(t2, o, sn)
                            nc.vector.tensor_mul(t3, e, sn)
                            nc.vector.tensor_mul(t4, o, cs)
                            halves = rot[:, off:off + DLOC].rearrange(
                                "p (h eo d) -> p h eo d", h=HPC, eo=2
                            )
                            h2 = lambda ap: ap.rearrange("p (h d) -> p h d", h=HPC)
                            nc.vector.tensor_sub(halves[:, :, 0, :], h2(t1), h2(t2))
                            nc.vector.tensor_add(halves[:, :, 1, :], h2(t3), h2(t4))
                        # transpose 4x [128,128]: q h0, q h1, k h0, k h1
                        tps = psmisc.tile([P, 4 * P], f32, tag="tps")
                        for g in range(4):
                            nc.tensor.transpose(
                                tps[:, g * P:(g + 1) * P],
                                rot[:, g * P:(g + 1) * P], ident,
                            )
                        tsl = slice(tt * P, (tt + 1) * P)
                        nc.any.tensor_copy(
                            qT[:, :, tsl],
                            tps[:, 0:2 * P].rearrange("p (h n) -> p h n", h=HPC),
                        )
                        nc.any.tensor_copy(
                            kT[:, :, tsl],
                            tps[:, 2 * P:4 * P].rearrange("p (h n) -> p h n", h=HPC),
                        )

                # ---- phase A: causal attention per head / q-block ----
                for h in range(HPC):
                    for qb in range(NQB):
                        qsl = slice(qb * QB, (qb + 1) * QB)
                        oT_ps = psot.tile([P, QB], f32, tag="ot")
                        nkt = 4 * qb + 4
                        acc = srp.tile([P, QB], f32, tag="acc")
                        for kt in range(nkt):
                            s_ps = pssc.tile([P, QB], f32, tag="s512")
                            nc.tensor.matmul(
                                s_ps, kT[:, h, kt * P:(kt + 1) * P],
                                qT[:, h, qsl], start=True, stop=True,
                            )
                            pT = ptp.tile([P, QB], f32r, tag="pT")
                            nc.scalar.activation(out=pT, in_=s_ps, func=EXP,
                                                 scale=SCALE)
                            a = kt - 4 * qb
                            if a >= 0:  # diagonal tile: causal mask
                                w = (a + 1) * P
                                nc.vector.tensor_mul(
                                    pT[:, 0:w], pT[:, 0:w],
                                    tri_sb[:, 512 - a * P:512 - a * P + w],
                                )
                            # softmax denominator partials on idle GpSimd
                            if kt == 0:
                                nc.gpsimd.tensor_copy(acc, pT.bitcast(f32))
                            else:
                                nc.gpsimd.tensor_add(acc, acc, pT.bitcast(f32))
                            nc.tensor.matmul(
                                oT_ps, vsb[:, kt, h * HD:(h + 1) * HD], pT,
                                start=(kt == 0), stop=(kt == nkt - 1),
                            )
                        sums = rsp.tile([1, QB], f32, tag="sums")
                        nc.gpsimd.tensor_reduce(
                            out=sums, in_=acc,
                            axis=mybir.AxisListType.C, op=mybir.AluOpType.add,
                        )
                        rs = rsp.tile([1, QB], f32, tag="rs")
                        nc.vector.reciprocal(rs, sums)
                        # broadcast across partitions via a DRAM round-trip
                        rsd = drp.tile([1, QB], f32, tag="rsd")
                        nc.gpsimd.dma_start(out=rsd, in_=rs)
                        rb_sb = rbp.tile([P, QB], f32, tag="rb")
                        nc.gpsimd.dma_start(out=rb_sb, in_=rsd.partition_broadcast(P))
                        nc.vector.tensor_mul(oT[:, h, qsl], oT_ps, rb_sb)

                # ---- phase W: output projection (row-sharded Wp partial) ----
                for tt in range(TB):
                    for co in range(C // 512):
                        y_ps = pssc.tile([P, 512], f32, tag="s512")
                        for h in range(HPC):
                            nc.tensor.matmul(
                                y_ps, oT[:, h, tt * P:(tt + 1) * P],
                                wp_sb[:, h, co * 512:(co + 1) * 512],
                                start=(h == 0), stop=(h == HPC - 1),
                            )
                        y_sb = ysbp.tile([P, 512], f32, tag="ysb")
                        nc.any.tensor_copy(y_sb, y_ps)
                        nc.sync.dma_start(
                            out=y[b * T + tt * P:b * T + (tt + 1) * P,
                                  co * 512:(co + 1) * 512],
                            in_=y_sb,
                        )

    return _split_excess_waits(nc)


def kernel(**inputs):
    global LAST_EXEC_NS
    _ensure_profile_shim()
    from concourse.bass_utils import run_bass_kernel_spmd

    x = np.asarray(inputs["x"], dtype=np.float32)
    Wq = np.asarray(inputs["Wq"], dtype=np.float32)
    Wk = np.asarray(inputs["Wk"], dtype=np.float32)
    Wv = np.asarray(inputs["Wv"], dtype=np.float32)
    Wp = np.asarray(inputs["Wp"], dtype=np.float32)
    rope_cos = np.asarray(inputs["rope_cos"], dtype=np.float32)
    rope_sin = np.asarray(inputs["rope_sin"], dtype=np.float32)

    xT = np.ascontiguousarray(x.reshape(NT, C).T)
    cos2 = np.ascontiguousarray(np.concatenate([rope_cos, rope_cos], axis=1))
    sin2 = np.ascontiguousarray(np.concatenate([rope_sin, rope_sin], axis=1))
    tri = np.zeros((P, 640), dtype=np.float32)
    ii = np.arange(P)
    tri[:, 512:] = (ii[None, :] >= ii[:, None]).astype(np.float32)

    in_maps = []
    for c in range(NCORE):
        rows = slice(c * DLOC, (c + 1) * DLOC)
        wqk_c = np.ascontiguousarray(
            np.concatenate([Wq[rows].T, Wk[rows].T], axis=1)
        )
        wv_c = np.ascontiguousarray(Wv[rows].T)
        wp_c = np.ascontiguousarray(Wp[:, rows].T)
        in_maps.append({
            "xT": xT, "wqk": wqk_c, "wv": wv_c, "wp": wp_c,
            "cos2": cos2, "sin2": sin2, "tri": tri,
        })

    if "nc" not in _cache:
        _cache["nc"] = _build_nc()
    res = run_bass_kernel_spmd(
        _cache["nc"], in_maps, core_ids=list(range(NCORE)), trace=TRACE,
    )
    LAST_EXEC_NS = res.exec_time_ns

    out = res.results[0]["y"].astype(np.float32)
    for c in range(1, NCORE):
        out += res.results[c]["y"]
    return out.reshape(B, T, C)



# revision 21
# speedup vs baseline: 3.9409x; 3.9409x over previous
"""Causal self-attention with RoPE on 8 Trainium2 NeuronCores — v2.

Sharding: tensor-parallel over heads (16 heads / 8 cores = 2 heads per
core). Each core computes q/k/v projections for its 2 heads over all
batches/tokens, runs causal attention locally, and applies its 256-row
slice of the output projection, producing a PARTIAL output [B*T, C].
The host sums the 8 partials (the all-reduce of the row-sharded Wp).

v2 changes vs v1 (3.34 ms):
  - The v1 bottleneck was the softmax denominator accumulated on GpSimd
    (Pool): tensor_add on [128,512] measured 7.5 us/op, 2.4 ms total,
    serializing the kernel. v2 accumulates exp partials on DVE (f32),
    then one ones-matmul on PE does partition-reduce + broadcast in a
    single 213 ns op, then reciprocal_approx_fast (custom DVE op) and a
    DVE multiply normalize. No GpSimd in the denominator path, no DRAM
    round-trip broadcast.
  - bf16 tiles everywhere (weights, x, q/k/v, P, output): DVE runs in
    4x perf mode on all-SBUF bf16 ops, DMA bytes halve, SBUF pressure
    halves. PE rate is unchanged (f32r was already 1 cyc/row at N>=256)
    so matmul precision loss is the only cost; tolerance is 2e-2 and
    the whole-pipeline bf16 error lands ~1e-2 rms, acceptable.
  - Host-side permutation of Wq/Wk columns to [head0 evens | head0 odds
    | head1 evens | head1 odds] makes RoPE a set of contiguous-block
    DVE ops (no stride-2 access patterns).
  - Software-pipelined emission: attention of batch b is interleaved,
    at ~426 ns granularity, with output-projection chains of batch b
    (as their q-blocks complete) and projection ci-steps of batch b+1,
    so the PE never idles while the Scalar engine runs exp (650 ns/tile
    vs PE's 426 ns/tile in the attention inner loop).
"""
import sys
import types
from collections import deque

sys.path.insert(0, "/opt/trn_rl_repo")

import numpy as np
import ml_dtypes

BF16 = ml_dtypes.bfloat16

B, T, C, H, HD = 4, 2048, 2048, 16, 128
P = 128
NCORE = 8
HPC = H // NCORE            # heads per core
DLOC = HPC * HD             # local feature width (256)
NT = B * T
KT = C // P                 # 16 contraction tiles
TB = T // P                 # 16 token tiles per batch
QB = 512                    # attention q-block width
NQB = T // QB
XBLK = 512                  # xT streaming block (tokens)
SCALE = float(1.0 / np.sqrt(HD))

LAST_EXEC_NS = None
TRACE = False
INTERLEAVE = True

_cache = {}


def _ensure_profile_shim():
    """antenv.axon_hooks is absent from the container stub; recreate it so
    run_bass_kernel_spmd(trace=True) can reach the NTFF profile hook."""
    import antenv

    if "antenv.axon_hooks" in sys.modules:
        return
    hooks = types.ModuleType("antenv.axon_hooks")
    hooks._hook = None
    hooks.set_axon_ntff_profile_hook = lambda h: setattr(hooks, "_hook", h)
    hooks.get_axon_ntff_profile_hook = lambda: hooks._hook
    sys.modules["antenv.axon_hooks"] = hooks
    antenv.axon_hooks = hooks
    try:
        from trn_agent_boot.trn_boot import _ntff_profile_via_ctypes

        hooks.set_axon_ntff_profile_hook(
            _ntff_profile_via_ctypes("/opt/axon/libaxon_pjrt.so")
        )
    except Exception:
        pass


def _split_excess_waits(nc):
    """HW instruction structs hold ONE sync wait (EventSemaphore: two), but
    Tile sometimes emits more. Hoist excess waits onto prefix NoOps."""
    import concourse.mybir as mybir

    uid = [0]
    for fn in nc.m.functions:
        for blk in fn.blocks:
            out = []
            for inst in blk.instructions:
                si = inst.sync_info
                waits = list(si.on_wait) if si and si.on_wait else []
                cap = 2 if inst.opcode == "EventSemaphore" else 1
                if len(waits) > cap:
                    keep = waits[-cap:]
                    for w in waits[:-cap]:
                        uid[0] += 1
                        out.append(
                            mybir.InstNoOp(
                                name=f"I-waitsplit-{uid[0]}",
                                engine=inst.engine,
                                text_hint="waitsplit",
                                sync_info=mybir.SyncInfo(on_wait=[w], on_update=[]),
                            )
                        )
                    si.on_wait = keep
                out.append(inst)
            blk.instructions = out
    return nc


def _build_nc():
    import concourse.bass as bass
    import concourse.mybir as mybir
    from concourse.tile import TileContext
    from contextlib import ExitStack

    f32 = mybir.dt.float32
    f32r = mybir.dt.float32r
    bf16 = mybir.dt.bfloat16
    EXP = mybir.ActivationFunctionType.Exp
    COPY = mybir.ActivationFunctionType.Copy

    nc = bass.Bass(trn_type="TRN2", target_bir_lowering=False)
    xT = nc.dram_tensor("xT", [C, NT], bf16, kind="ExternalInput")
    wqk = nc.dram_tensor("wqk", [C, 2 * DLOC], bf16, kind="ExternalInput")
    wv = nc.dram_tensor("wv", [C, DLOC], bf16, kind="ExternalInput")
    wp = nc.dram_tensor("wp", [DLOC, C], bf16, kind="ExternalInput")
    cos4 = nc.dram_tensor("cos4", [T, 2 * P], bf16, kind="ExternalInput")
    sin4 = nc.dram_tensor("sin4", [T, 2 * P], bf16, kind="ExternalInput")
    tri = nc.dram_tensor("tri", [P, 640], bf16, kind="ExternalInput")
    ident_d = nc.dram_tensor("ident", [P, P], f32r, kind="ExternalInput")
    ones_d = nc.dram_tensor("ones", [P, P], f32r, kind="ExternalInput")
    y = nc.dram_tensor("y", [NT, C], bf16, kind="ExternalOutput")

    with nc.allow_low_precision(
        reason="bf16 pipeline; tolerance 2e-2 with ~1e-2 expected error"
    ), TileContext(nc) as tc:
        stk = ExitStack()
        wpool = stk.enter_context(tc.tile_pool(name="wpool", bufs=1))
        bpool = stk.enter_context(tc.tile_pool(name="bpool", bufs=2))
        xpool = stk.enter_context(tc.tile_pool(name="xpool", bufs=2))
        qspool = stk.enter_context(tc.tile_pool(name="qspool", bufs=2))
        tpool = stk.enter_context(tc.tile_pool(name="tpool", bufs=2))
        rotp = stk.enter_context(tc.tile_pool(name="rotp", bufs=2))
        ptp = stk.enter_context(tc.tile_pool(name="ptp", bufs=3))
        accp = stk.enter_context(tc.tile_pool(name="accp", bufs=2))
        rbp = stk.enter_context(tc.tile_pool(name="rbp", bufs=2))
        ysbp = stk.enter_context(tc.tile_pool(name="ysbp", bufs=4))
        psqk = stk.enter_context(tc.tile_pool(name="psqk", bufs=2, space="PSUM"))
        psaux = stk.enter_context(tc.tile_pool(name="psaux", bufs=2, space="PSUM"))
        pssc = stk.enter_context(tc.tile_pool(name="pssc", bufs=2, space="PSUM"))
        psot = stk.enter_context(tc.tile_pool(name="psot", bufs=2, space="PSUM"))
        with stk:
            # ---- weights / constants (split DMAs to spread across queues) ----
            wqk_sb = wpool.tile([P, KT, 2 * DLOC], bf16, tag="wqk")
            for g in range(4):
                nc.sync.dma_start(
                    out=wqk_sb[:, 4 * g:4 * g + 4, :],
                    in_=wqk[g * 512:(g + 1) * 512, :].rearrange(
                        "(t p) m -> p t m", p=P),
                )
            wv_sb = wpool.tile([P, KT, DLOC], bf16, tag="wv")
            for g in range(4):
                nc.sync.dma_start(
                    out=wv_sb[:, 4 * g:4 * g + 4, :],
                    in_=wv[g * 512:(g + 1) * 512, :].rearrange(
                        "(t p) m -> p t m", p=P),
                )
            wp_sb = wpool.tile([P, HPC, C], bf16, tag="wp")
            for h in range(HPC):
                nc.sync.dma_start(
                    out=wp_sb[:, h, :], in_=wp[h * P:(h + 1) * P, :])
            cos_sb = wpool.tile([P, TB, 2 * P], bf16, tag="cos")
            sin_sb = wpool.tile([P, TB, 2 * P], bf16, tag="sin")
            for g in range(2):
                nc.sync.dma_start(
                    out=cos_sb[:, 8 * g:8 * g + 8, :],
                    in_=cos4[g * 1024:(g + 1) * 1024, :].rearrange(
                        "(t p) d -> p t d", p=P),
                )
                nc.sync.dma_start(
                    out=sin_sb[:, 8 * g:8 * g + 8, :],
                    in_=sin4[g * 1024:(g + 1) * 1024, :].rearrange(
                        "(t p) d -> p t d", p=P),
                )
            tri_sb = wpool.tile([P, 640], bf16, tag="tri")
            nc.sync.dma_start(out=tri_sb, in_=tri[:, :])
            ident = wpool.tile([P, P], f32r, tag="ident")
            nc.sync.dma_start(out=ident, in_=ident_d[:, :])
            ones_sb = wpool.tile([P, P], f32r, tag="ones")
            nc.sync.dma_start(out=ones_sb, in_=ones_d[:, :])

            # per-batch tiles, double-buffered for cross-batch overlap
            def batch_tiles():
                qT = bpool.tile([P, HPC, T], bf16, tag="qT")
                kT = bpool.tile([P, HPC, T], bf16, tag="kT")
                vsb = bpool.tile([P, TB, DLOC], bf16, tag="v")
                oT = bpool.tile([P, HPC, T], bf16, tag="oT")
                return qT, kT, vsb, oT

            tiles = {}

            # ---------- phase P: projections + rope + transposes ----------
            def p_phase(b):
                """Generator: yields after each ci-step (~320 ns of PE)."""
                tiles[b] = batch_tiles()
                qT, kT, vsb, oT = tiles[b]
                pending_tp = [None]

                for blk in range(T // XBLK):
                    xt = xpool.tile([P, KT, XBLK], bf16, tag="xt")
                    col0 = b * T + blk * XBLK
                    for g in range(8):
                        nc.sync.dma_start(
                            out=xt[:, 2 * g:2 * g + 2, :],
                            in_=xT[g * 256:(g + 1) * 256,
                                   col0:col0 + XBLK].rearrange(
                                "(t p) n -> p t n", p=P),
                        )
                    for st in range(XBLK // P):
                        tt = (blk * XBLK) // P + st
                        xts = xt[:, :, st * P:(st + 1) * P]
                        ps_qk = psqk.tile([P, 512], f32, tag="qk")
                        for ci in range(KT):
                            nc.tensor.matmul(
                                ps_qk, xts[:, ci, :], wqk_sb[:, ci, :],
                                start=(ci == 0), stop=(ci == KT - 1),
                            )
                            if ci == 3 and pending_tp[0] is not None:
                                pending_tp[0]()
                                pending_tp[0] = None
                            yield

                        # v projection as one atomic burst (psaux slot must
                        # not stay open across yields: a den/tp alloc taking
                        # the ring slot mid-chain would deadlock the PE)
                        ps_v = psaux.tile([P, 512], f32, tag="aux")
                        for ci in range(KT):
                            nc.tensor.matmul(
                                ps_v[:, 0:DLOC], xts[:, ci, :], wv_sb[:, ci, :],
                                start=(ci == 0), stop=(ci == KT - 1),
                            )
                        nc.scalar.activation(out=vsb[:, tt, :],
                                             in_=ps_v[:, 0:DLOC], func=COPY)
                        # stage to SBUF (scalar), rope on DVE (all-bf16 4x)
                        qs = qspool.tile([P, 512], bf16, tag="qs")
                        nc.scalar.activation(out=qs, in_=ps_qk, func=COPY)
                        qsv = qs.rearrange("p (g x d) -> p g x d", g=4, x=2)
                        e = qsv[:, :, 0, :]
                        o = qsv[:, :, 1, :]
                        cs = cos_sb[:, tt, :].rearrange("p (g d) -> p g d", g=4)
                        sn = sin_sb[:, tt, :].rearrange("p (g d) -> p g d", g=4)
                        t1 = tpool.tile([P, 4, 64], bf16, tag="t1")
                        t2 = tpool.tile([P, 4, 64], bf16, tag="t2")
                        t3 = tpool.tile([P, 4, 64], bf16, tag="t3")
                        t4 = tpool.tile([P, 4, 64], bf16, tag="t4")
                        nc.vector.tensor_mul(t1, e, cs)
                        nc.vector.tensor_mul(t2, o, sn)
                        nc.vector.tensor_mul(t3, e, sn)
                        nc.vector.tensor_mul(t4, o, cs)
                        rot = rotp.tile([P, 512], f32r, tag="rot")
                        rv = rot.rearrange("p (g x d) -> p g x d", g=4, x=2)
                        nc.vector.tensor_sub(rv[:, :, 0, :], t1, t2)
                        nc.vector.tensor_add(rv[:, :, 1, :], t3, t4)

                        def tp_stage(rot=rot, tt=tt, qT=qT, kT=kT):
                            tp = psaux.tile([P, 512], f32r, tag="aux")
                            for g in range(4):
                                nc.tensor.transpose(
                                    tp[:, g * P:(g + 1) * P],
                                    rot[:, g * P:(g + 1) * P], ident,
                                )
                            tsl = slice(tt * P, (tt + 1) * P)
                            nc.vector.tensor_copy(
                                qT[:, :, tsl],
                                tp[:, 0:2 * P].rearrange(
                                    "p (h n) -> p h n", h=HPC),
                            )
                            nc.scalar.activation(
                                out=kT[:, :, tsl],
                                in_=tp[:, 2 * P:4 * P].rearrange(
                                    "p (h n) -> p h n", h=HPC),
                                func=COPY,
                            )

                        pending_tp[0] = tp_stage
                        yield
                if pending_tp[0] is not None:
                    pending_tp[0]()
                    pending_tp[0] = None

            # ---------- phase W: output projection chains (fillers) ----------
            ycopy_flip = [0]

            def w_chain(b, tt, co):
                # y_ps lives in the scores ring (short holds only there);
                # psqk slots are held across the whole interleaved qk chain
                # and sharing them would deadlock the PE.
                _, _, _, oT = tiles[b]
                y_ps = pssc.tile([P, 512], f32, tag="sc")
                for h in range(HPC):
                    nc.tensor.matmul(
                        y_ps, oT[:, h, tt * P:(tt + 1) * P],
                        wp_sb[:, h, co * 512:(co + 1) * 512],
                        start=(h == 0), stop=(h == HPC - 1),
                    )
                if co == 0:
                    ysb = ysbp.tile([P, C], bf16, tag="ysb")
                    tiles[("ysb", b, tt)] = ysb
                else:
                    ysb = tiles[("ysb", b, tt)]
                dst = ysb[:, co * 512:(co + 1) * 512]
                if ycopy_flip[0] % 2 == 0:
                    nc.scalar.activation(out=dst, in_=y_ps, func=COPY)
                else:
                    nc.vector.tensor_copy(dst, y_ps)
                ycopy_flip[0] += 1
                if co == 3:
                    del tiles[("ysb", b, tt)]
                    r0 = b * T + tt * P
                    nc.sync.dma_start(out=y[r0:r0 + P, 0:1024],
                                      in_=ysb[:, 0:1024])
                    nc.sync.dma_start(out=y[r0:r0 + P, 1024:2048],
                                      in_=ysb[:, 1024:2048])

            # ---------- phase A: causal attention ----------
            def a_phase(b, w_fill):
                """Generator: yields after each kt step (~426 ns of PE)."""
                qT, kT, vsb, oT = tiles[b]
                for qb in range(NQB):
                    for h in range(HPC):
                        qsl = slice(qb * QB, (qb + 1) * QB)
                        oT_ps = psot.tile([P, QB], f32, tag="ot")
                        acc = accp.tile([P, QB], f32r, tag="acc")
                        nkt = 4 * qb + 4
                        for kt in range(nkt):
                            s_ps = pssc.tile([P, QB], f32, tag="sc")
                            nc.tensor.matmul(
                                s_ps, kT[:, h, kt * P:(kt + 1) * P],
                                qT[:, h, qsl], start=True, stop=True,
                            )
                            pT = ptp.tile([P, QB], bf16, tag="pT")
                            nc.scalar.activation(out=pT, in_=s_ps, func=EXP,
                                                 scale=SCALE)
                            a = kt - 4 * qb
                            if a >= 0:  # block-diagonal tile: causal mask
                                w = (a + 1) * P
                                nc.vector.tensor_mul(
                                    pT[:, 0:w], pT[:, 0:w],
                                    tri_sb[:, 512 - a * P:512 - a * P + w],
                                )
                            if kt == 0:
                                nc.vector.tensor_copy(acc, pT)
                            else:
                                nc.vector.tensor_add(acc, acc, pT)
                            nc.tensor.matmul(
                                oT_ps, vsb[:, kt, h * HD:(h + 1) * HD], pT,
                                start=(kt == 0), stop=(kt == nkt - 1),
                            )
                            yield
                        # denominator: partition-reduce+broadcast via
                        # ones-matmul, then fast reciprocal and normalize
                        den = psaux.tile([P, QB], f32, tag="aux")
                        nc.tensor.matmul(
                            den, ones_sb, acc, start=True, stop=True,
                        )
                        rb = rbp.tile([P, QB], f32, tag="rb")
                        nc.vector.reciprocal(rb, den)
                        nc.vector.tensor_mul(oT[:, h, qsl], oT_ps, rb)
                    # q-block complete for both heads -> out-proj fillers
                    for tt in range(qb * 4, qb * 4 + 4):
                        for co in range(4):
                            w_fill.append(
                                lambda b=b, tt=tt, co=co: w_chain(b, tt, co))

            # ---------- software-pipelined emission ----------
            w_fill = deque()
            for _ in p_phase(0):
                pass
            for b in range(B):
                ap = a_phase(b, w_fill)
                pp = p_phase(b + 1) if b + 1 < B else None
                cnt = 0
                for _ in ap:
                    cnt += 1
                    if INTERLEAVE and cnt % 3 != 0:  # 2 fillers / 3 A-steps
                        if w_fill:
                            w_fill.popleft()()
                        elif pp is not None:
                            if next(pp, _SENT) is _SENT:
                                pp = None
                if pp is not None:
                    for _ in pp:
                        pass
                while w_fill:
                    w_fill.popleft()()

    return _split_excess_waits(nc)


_SENT = object()


def kernel(**inputs):
    global LAST_EXEC_NS
    _ensure_profile_shim()
    from concourse.bass_utils import run_bass_kernel_spmd

    x = np.asarray(inputs["x"], dtype=np.float32)
    Wq = np.asarray(inputs["Wq"], dtype=np.float32)
    Wk = np.asarray(inputs["Wk"], dtype=np.float32)
    Wv = np.asarray(inputs["Wv"], dtype=np.float32)
    Wp = np.asarray(inputs["Wp"], dtype=np.float32)
    rope_cos = np.asarray(inputs["rope_cos"], dtype=np.float32)
    rope_sin = np.asarray(inputs["rope_sin"], dtype=np.float32)

    xT = np.ascontiguousarray(x.reshape(NT, C).T.astype(BF16))
    cos4 = np.ascontiguousarray(np.tile(rope_cos, (1, 4)).astype(BF16))
    sin4 = np.ascontiguousarray(np.tile(rope_sin, (1, 4)).astype(BF16))
    tri = np.zeros((P, 640), dtype=np.float32)
    ii = np.arange(P)
    tri[:, 512:] = (ii[None, :] >= ii[:, None]).astype(np.float32)
    tri = tri.astype(BF16)

    # per-head column permutation: [evens(64) | odds(64)] per head so rope
    # reads/writes contiguous blocks on-device
    perm = np.concatenate(
        [h * HD + np.concatenate([np.arange(0, HD, 2), np.arange(1, HD, 2)])
         for h in range(HPC)]
    )

    in_maps = []
    for c in range(NCORE):
        rows = slice(c * DLOC, (c + 1) * DLOC)
        wq_c = Wq[rows][perm].T
        wk_c = Wk[rows][perm].T
        wqk_c = np.ascontiguousarray(
            np.concatenate([wq_c, wk_c], axis=1).astype(BF16))
        wv_c = np.ascontiguousarray(Wv[rows].T.astype(BF16))
        wp_c = np.ascontiguousarray(Wp[:, rows].T.astype(BF16))
        in_maps.append({
            "xT": xT, "wqk": wqk_c, "wv": wv_c, "wp": wp_c,
            "cos4": cos4, "sin4": sin4, "tri": tri,
            "ident": np.eye(P, dtype=np.float32),
            "ones": np.ones((P, P), dtype=np.float32),
        })

    if "nc" not in _cache:
        _cache["nc"] = _build_nc()
    res = run_bass_kernel_spmd(
        _cache["nc"], in_maps, core_ids=list(range(NCORE)), trace=TRACE,
    )
    LAST_EXEC_NS = res.exec_time_ns

    out = res.results[0]["y"].astype(np.float32)
    for c in range(1, NCORE):
        out += res.results[c]["y"].astype(np.float32)
    return out.reshape(B, T, C)
